# revision 1
# baseline (speedup 1.0000x reference)
"""GraphMAE-style GNN forward (3x GCNConv + BN + PReLU + SCE loss) on 8 TRN2
NeuronCores via Bass/Tile.

Sharding: nodes partitioned across 8 cores (6272 rows each, node space padded
to 50176). Per layer: sharded dense matmul (feature-major) -> row scale by
dinv -> AllGather of the bf16 node-major table -> per-dst-tile edge
aggregation via indirect-DMA row gathers + one-hot segment-sum matmuls
accumulated in PSUM (self-loops folded in as identity matmuls on the local
shard tile) -> BatchNorm batch stats via free-dim reductions + AllReduce ->
fused BN affine + PReLU (as max(t, a*t), valid for 0<a<=1). Loss: per-core
masked-row cosine-similarity partial sums, AllReduce, 1 - sum/NMASK.

Per-feature conv bias is skipped: training-mode BatchNorm subtracts the batch
mean, so the bias cancels exactly.
"""
import sys

sys.path.insert(0, "/opt/trn_rl_repo")
import numpy as np
import ml_dtypes

import concourse.bass as bass
import concourse.mybir as mybir
import concourse.tile as tile
from bass_rust import SyncInfo
from concourse.bass import IndirectOffsetOnAxis
from concourse.bass_utils import run_bass_kernel_spmd
from concourse.tile_rust import add_dep_helper
from concourse.vector_clock import ScopedClock

BF = ml_dtypes.bfloat16
F32 = mybir.dt.float32
BF16 = mybir.dt.bfloat16
I32 = mybir.dt.int32
OP = mybir.AluOpType

N, E, IN, HID, OUT = 50000, 800000, 128, 256, 128
NC, P = 8, 128
SHARD, T = 6272, 49
NPAD = NC * SHARD
NMASK = 25000
EPS = 1e-5
RG = [list(range(NC))]

# ---------------------------------------------------------------------------
# Walrus on this toolchain rejects >1 semaphore wait per instruction
# ("Too many sync wait commands"). Post-process lowered blocks: hoist excess
# waits onto injected same-engine NoOps (program order => equivalent).
_uid = [0]


def _split_bb_waits(nc):
    for f in nc.m.functions:
        for bb in f.blocks:
            insts = list(bb.instructions)
            out = []
            changed = False
            for inst in insts:
                si = inst.sync_info
                waits = list(si.on_wait) if si is not None and si.on_wait else []
                if len(waits) > 1:
                    changed = True
                    rest = waits[:-1]
                    inst.sync_info.on_wait = waits[-1:]
                    while rest:
                        _uid[0] += 1
                        nop = mybir.InstNoOp(
                            name=f"I-waitsplit-{_uid[0]}", ins=[], outs=[])
                        nop.engine = inst.engine
                        nop.sync_info = SyncInfo(
                            on_wait=rest[:1], on_update=[])
                        rest = rest[1:]
                        out.append(nop)
                out.append(inst)
            if changed:
                bb.instructions = out


class TileContextSplitDrain(tile.TileContext):
    def _drain_and_barrier(self, tick_clock, wait_clock):
        nc = self.nc
        probe = nc.sync.nop(nofuse=True)
        wait_clock.add_sem_waits(
            probe.ins, ScopedClock({None: tick_clock.global_clock}))
        nc.sync.drain()
        nc.all_engine_barrier()
        assert self.sems is not None
        popped = nc._tile_sem_poison_stack.pop()
        assert popped is self._sem_poison
        nc.clear_and_free_semaphores(list(self.sems.allocated().values()))
        nc.all_engine_barrier()
        _split_bb_waits(nc)


# ---------------------------------------------------------------------------
def _prep(edge_index, mask_nodes):
    src = edge_index[0].astype(np.int64)
    dst = edge_index[1].astype(np.int64)
    deg = np.bincount(dst, minlength=N).astype(np.float64) + 1.0
    dinv = (1.0 / np.sqrt(deg)).astype(np.float32)
    dinv_pad = np.zeros(NPAD, np.float32)
    dinv_pad[:N] = dinv

    mask_set = np.zeros(N, bool)
    mask_set[mask_nodes] = True

    pertile = []
    kc = 0
    core_of = dst // SHARD
    for c in range(NC):
        sel = core_of == c
        es, ed = src[sel], dst[sel] - c * SHARD
        tl = ed // P
        order = np.argsort(tl, kind="stable")
        es, ed, tl = es[order], ed[order], tl[order]
        tiles = []
        for t in range(T):
            m = tl == t
            tiles.append((es[m], (ed[m] % P).astype(np.float32)))
            kc = max(kc, (int(m.sum()) + P - 1) // P)
        pertile.append(tiles)

    srcs, dstl = [], []
    for c in range(NC):
        sc = np.zeros((T * kc, P), np.int32)
        dc = np.full((T * kc, P), 255.0, np.float32)
        for t in range(T):
            es, dl = pertile[c][t]
            n = len(es)
            sc[t * kc:(t + 1) * kc].reshape(-1)[:n] = es
            dc[t * kc:(t + 1) * kc].reshape(-1)[:n] = dl
        srcs.append(np.ascontiguousarray(sc.T))
        dstl.append(np.ascontiguousarray(dc.T.astype(BF)))

    mlocal, mglob, mvalid, mkeep = [], [], [], []
    locs = []
    mc = 0
    for c in range(NC):
        rows = np.arange(c * SHARD, (c + 1) * SHARD)
        valid = rows < N
        mm = np.zeros(SHARD, bool)
        mm[:valid.sum()] = mask_set[rows[valid]]
        loc = np.where(mm)[0]
        mc = max(mc, (len(loc) + P - 1) // P)
        locs.append(loc)
        keep = np.ones(SHARD, np.float32)
        keep[mm] = 0.0
        mkeep.append(keep)
    for c in range(NC):
        loc = locs[c]
        n = len(loc)
        lo = np.zeros((mc, P), np.int32)
        gl = np.zeros((mc, P), np.int32)
        va = np.zeros((mc, P), np.float32)
        lo.reshape(-1)[:n] = loc
        gl.reshape(-1)[:n] = loc + c * SHARD
        va.reshape(-1)[:n] = 1.0
        mlocal.append(np.ascontiguousarray(lo.T))
        mglob.append(np.ascontiguousarray(gl.T))
        mvalid.append(np.ascontiguousarray(va.T))
    return dinv_pad, srcs, dstl, kc, mlocal, mglob, mvalid, mkeep, mc


def build_nc(kc, mc):
    nc = bass.Bass(num_devices=NC, target_bir_lowering=False)
    D = {}

    def inp(name, shape, dt):
        D[name] = nc.dram_tensor(name, shape, dt, kind="ExternalInput")
        return D[name]

    x_shard = inp("x_shard", [SHARD, IN], F32)
    x_full = inp("x_full", [N, IN], F32)
    inp("src_rows", [P, T * kc], I32)
    inp("dst_local", [P, T * kc], BF16)
    inp("dinv_rep", [P, SHARD], F32)
    inp("mcol", [P, T], F32)
    inp("token_rep", [P, IN], F32)
    inp("iota_bf", [P, P], BF16)
    inp("ident_bf", [P, P], BF16)
    inp("ident_f32", [P, P], F32)
    inp("ones_col", [P, 1], F32)
    inp("w1", [IN, HID], BF16)
    inp("w2a", [P, OUT], BF16)
    inp("w2b", [P, OUT], BF16)
    inp("wd", [OUT, IN], BF16)
    inp("gb", [P, 8], F32)
    inp("a_rep", [P, 3], F32)
    inp("mrow_l", [P, mc], I32)
    inp("mrow_g", [P, mc], I32)
    inp("mval", [P, mc], F32)
    out_t = nc.dram_tensor("loss", [1, 1], F32, kind="ExternalOutput")

    with TileContextSplitDrain(nc) as tc:
        with (
            tc.tile_pool(name="const", bufs=1) as cpool,
            tc.tile_pool(name="hbuf", bufs=1) as hpool,
            tc.tile_pool(name="vbuf", bufs=1) as vpool,
            tc.tile_pool(name="hwn", bufs=1) as hwpool,
            tc.tile_pool(name="work", bufs=2) as wk,
            tc.tile_pool(name="gpool", bufs=4) as gp,
            tc.tile_pool(name="psmm", bufs=2, space="PSUM") as psmm,
            tc.tile_pool(name="pstr", bufs=1, space="PSUM") as pstr,
            tc.tile_pool(name="psagg", bufs=1, space="PSUM") as psagg,
            tc.tile_pool(name="dram", bufs=1, space="DRAM") as dpool,
        ):
            def load(tname):
                h = D[tname]
                t_ = cpool.tile(list(h.shape), h.dtype, tag=tname, name=tname)
                nc.sync.dma_start(t_[:], h[:])
                return t_

            src_s = load("src_rows")
            dstl_s = load("dst_local")
            dinv_s = load("dinv_rep")
            mcol_s = load("mcol")
            tokr_s = load("token_rep")
            iota_s = load("iota_bf")
            idbf_s = load("ident_bf")
            idf_s = load("ident_f32")
            ones_s = load("ones_col")
            w1_s = load("w1")
            w2a_s = load("w2a")
            w2b_s = load("w2b")
            wd_s = load("wd")
            gb_s = load("gb")
            a_s = load("a_rep")
            mrl_s = load("mrow_l")
            mrg_s = load("mrow_g")
            mv_s = load("mval")

            # h0: load x shard, mask in node-major, transpose to f-major bf16
            h0 = hpool.tile([P, SHARD], BF16, tag="hA_0", name="hA_0")
            for b in range(T):
                xt = wk.tile([P, IN], F32, tag="xt", name="xt")
                nc.sync.dma_start(xt[:], x_shard[b * P:(b + 1) * P, :])
                nc.vector.tensor_tensor(out=xt[:], in0=xt[:], in1=tokr_s[:],
                                        op=OP.subtract)
                nc.vector.tensor_scalar(out=xt[:], in0=xt[:],
                                        scalar1=mcol_s[:, b:b + 1], scalar2=None,
                                        op0=OP.mult)
                nc.vector.tensor_tensor(out=xt[:], in0=xt[:], in1=tokr_s[:],
                                        op=OP.add)
                pt = pstr.tile([P, P], F32, tag="ptr32", name="ptr32")
                nc.tensor.transpose(pt[:], xt[:], idf_s[:])
                nc.vector.tensor_copy(out=h0[:, b * P:(b + 1) * P], in_=pt[:])

            table1 = dpool.tile([NPAD, HID], BF16, addr_space="Shared", name="table1")
            table2 = dpool.tile([NPAD, OUT], BF16, addr_space="Shared", name="table2")
            table3 = dpool.tile([NPAD, IN], BF16, addr_space="Shared", name="table3")
            agin1 = dpool.tile([SHARD, HID], BF16, name="agin1")
            agin2 = dpool.tile([SHARD, OUT], BF16, name="agin2")
            agin3 = dpool.tile([SHARD, IN], BF16, name="agin3")
            loss_in = dpool.tile([1, 16], F32, name="loss_in")
            loss_out = dpool.tile([1, 16], F32, addr_space="Shared", name="loss_out")
            rex_dram = dpool.tile([SHARD, IN], F32, name="rex_dram")

            def layer(h_tiles, w_tiles, f_in, f_out, agin, table, g_col,
                      be_col, a_col, otag, mask_hwn):
                nfh = f_out // P
                nkt = f_in // P
                hwn = [hwpool.tile([P, f_out], BF16, tag=f"hwn_{b}", name=f"hwn_{b}")
                       for b in range(T)]
                for b in range(T):
                    for fh in range(nfh):
                        pm = psmm.tile([P, P], F32, tag="pm", name="pm")
                        for kt in range(nkt):
                            nc.tensor.matmul(
                                pm[:], lhsT=w_tiles[kt][:, fh * P:(fh + 1) * P],
                                rhs=h_tiles[kt][:, b * P:(b + 1) * P],
                                start=(kt == 0), stop=(kt == nkt - 1))
                        hwT = wk.tile([P, P], BF16, tag="hwT", name="hwT")
                        nc.vector.tensor_tensor(
                            out=hwT[:], in0=pm[:],
                            in1=dinv_s[:, b * P:(b + 1) * P], op=OP.mult)
                        ptb = pstr.tile([P, P], BF16, tag="ptrbf", name="ptrbf")
                        nc.tensor.transpose(ptb[:], hwT[:], idbf_s[:])
                        sl = hwn[b][:, fh * P:(fh + 1) * P]
                        if mask_hwn:
                            nc.vector.tensor_scalar(
                                out=sl, in0=ptb[:], scalar1=mcol_s[:, b:b + 1],
                                scalar2=None, op0=OP.mult)
                        else:
                            nc.vector.tensor_copy(out=sl, in_=ptb[:])
                    nc.sync.dma_start(agin[b * P:(b + 1) * P, :], hwn[b][:])
                cc = nc.gpsimd.collective_compute(
                    "AllGather", OP.bypass, replica_groups=RG,
                    ins=[agin[:].opt()], outs=[table[:].opt()])
                fence = nc.gpsimd.memset(
                    wk.tile([1, 1], F32, tag="fence", name="fence")[:], 0.0)
                add_dep_helper(fence.ins, cc.ins, True, "fence cc")

                v_tiles = [vpool.tile([P, SHARD], BF16, tag=f"v_{fh}", name=f"v_{fh}")
                           for fh in range(nfh)]
                s_cols = [wk.tile([P, T], F32, tag=f"sc_{fh}", name=f"sc_{fh}")
                          for fh in range(nfh)]
                q_cols = [wk.tile([P, T], F32, tag=f"qc_{fh}", name=f"qc_{fh}")
                          for fh in range(nfh)]
                for t in range(T):
                    pa = [psagg.tile([P, P], F32, tag=f"pa{fh}", name=f"pa{fh}")
                          for fh in range(nfh)]
                    for fh in range(nfh):
                        nc.tensor.matmul(
                            pa[fh][:], lhsT=hwn[t][:, fh * P:(fh + 1) * P],
                            rhs=idbf_s[:], start=True, stop=False)
                    for k in range(kc):
                        j = t * kc + k
                        G = gp.tile([P, f_out], BF16, tag="G", name="G")
                        nc.gpsimd.indirect_dma_start(
                            out=G[:], out_offset=None, in_=table[:],
                            in_offset=IndirectOffsetOnAxis(
                                ap=src_s[:, j:j + 1], axis=0))
                        S = gp.tile([P, P], BF16, tag="S", name="S")
                        nc.vector.tensor_tensor(
                            out=S[:],
                            in0=dstl_s[:, j:j + 1].to_broadcast([P, P]),
                            in1=iota_s[:], op=OP.is_equal)
                        for fh in range(nfh):
                            nc.tensor.matmul(
                                pa[fh][:], lhsT=G[:, fh * P:(fh + 1) * P],
                                rhs=S[:], start=False, stop=(k == kc - 1))
                    for fh in range(nfh):
                        vt = v_tiles[fh]
                        sl = slice(t * P, (t + 1) * P)
                        nc.vector.tensor_tensor(
                            out=vt[:, sl], in0=pa[fh][:],
                            in1=dinv_s[:, sl], op=OP.mult)
                        nc.vector.tensor_reduce(
                            out=s_cols[fh][:, t:t + 1], in_=vt[:, sl],
                            axis=mybir.AxisListType.X, op=OP.add)
                        sq = wk.tile([P, P], F32, tag="sq", name="sq")
                        nc.vector.tensor_tensor(
                            out=sq[:], in0=vt[:, sl], in1=vt[:, sl],
                            op=OP.mult)
                        nc.vector.tensor_reduce(
                            out=q_cols[fh][:, t:t + 1], in_=sq[:],
                            axis=mybir.AxisListType.X, op=OP.add)

                stats_in = dpool.tile([P, 4], F32, name=f"stats_in_{otag}")
                stats_out = dpool.tile([P, 4], F32, addr_space="Shared",
                                       name=f"stats_out_{otag}")
                st = wk.tile([P, 4], F32, tag="stats", name="stats")
                nc.gpsimd.memset(st[:], 0.0)
                for fh in range(nfh):
                    nc.vector.tensor_reduce(
                        out=st[:, fh:fh + 1], in_=s_cols[fh][:],
                        axis=mybir.AxisListType.X, op=OP.add)
                    nc.vector.tensor_reduce(
                        out=st[:, 2 + fh:3 + fh], in_=q_cols[fh][:],
                        axis=mybir.AxisListType.X, op=OP.add)
                nc.sync.dma_start(stats_in[:], st[:])
                cc2 = nc.gpsimd.collective_compute(
                    "AllReduce", OP.add, replica_groups=RG,
                    ins=[stats_in[:].opt()], outs=[stats_out[:].opt()])
                st2 = wk.tile([P, 4], F32, tag="stats2", name="stats2")
                ld2 = nc.sync.dma_start(st2[:], stats_out[:])
                add_dep_helper(ld2.ins, cc2.ins, True, "stats after ar")
                h_out = [hpool.tile([P, SHARD], BF16, tag=f"{otag}_{fh}", name=f"{otag}_{fh}")
                         for fh in range(nfh)]
                AB = []
                for fh in range(nfh):
                    mu = wk.tile([P, 1], F32, tag=f"mu{fh}", name=f"mu{fh}")
                    nc.vector.tensor_scalar(out=mu[:], in0=st2[:, fh:fh + 1],
                                            scalar1=1.0 / N, scalar2=None,
                                            op0=OP.mult)
                    var = wk.tile([P, 1], F32, tag=f"var{fh}", name=f"var{fh}")
                    nc.vector.tensor_tensor(out=var[:], in0=mu[:], in1=mu[:],
                                            op=OP.mult)
                    ms = wk.tile([P, 1], F32, tag=f"ms{fh}", name=f"ms{fh}")
                    nc.vector.tensor_scalar(out=ms[:], in0=st2[:, 2 + fh:3 + fh],
                                            scalar1=1.0 / N, scalar2=None,
                                            op0=OP.mult)
                    nc.vector.tensor_tensor(out=var[:], in0=ms[:], in1=var[:],
                                            op=OP.subtract)
                    nc.vector.tensor_scalar(out=var[:], in0=var[:], scalar1=EPS,
                                            scalar2=None, op0=OP.add)
                    rs = wk.tile([P, 1], F32, tag=f"rs{fh}", name=f"rs{fh}")
                    nc.scalar.activation(rs[:], var[:],
                                         mybir.ActivationFunctionType.Sqrt)
                    nc.vector.reciprocal(rs[:], rs[:])
                    A = wk.tile([P, 1], F32, tag=f"A{fh}", name=f"A{fh}")
                    nc.vector.tensor_tensor(out=A[:], in0=g_col[fh], in1=rs[:],
                                            op=OP.mult)
                    Bv = wk.tile([P, 1], F32, tag=f"B{fh}", name=f"B{fh}")
                    nc.vector.tensor_tensor(out=Bv[:], in0=mu[:], in1=A[:],
                                            op=OP.mult)
                    nc.vector.tensor_tensor(out=Bv[:], in0=be_col[fh], in1=Bv[:],
                                            op=OP.subtract)
                    AB.append((A, Bv))
                for t in range(T):
                    for fh in range(nfh):
                        A, Bv = AB[fh]
                        sl = slice(t * P, (t + 1) * P)
                        t1 = wk.tile([P, P], F32, tag="t1", name="t1")
                        nc.vector.tensor_scalar(
                            out=t1[:], in0=v_tiles[fh][:, sl],
                            scalar1=A[:, :1], scalar2=Bv[:, :1],
                            op0=OP.mult, op1=OP.add)
                        t2 = wk.tile([P, P], F32, tag="t2", name="t2")
                        nc.vector.tensor_scalar(
                            out=t2[:], in0=t1[:], scalar1=a_col, scalar2=None,
                            op0=OP.mult)
                        nc.vector.tensor_tensor(
                            out=h_out[fh][:, sl], in0=t1[:], in1=t2[:],
                            op=OP.max)
                return h_out

            h1 = layer([h0], [w1_s], IN, HID, agin1, table1,
                       [gb_s[:, 0:1], gb_s[:, 1:2]],
                       [gb_s[:, 4:5], gb_s[:, 5:6]], a_s[:, 0:1], "hB", False)
            h2 = layer(h1, [w2a_s, w2b_s], HID, OUT, agin2, table2,
                       [gb_s[:, 2:3]], [gb_s[:, 6:7]], a_s[:, 1:2], "hA",
                       False)
            h3 = layer(h2, [wd_s], OUT, IN, agin3, table3,
                       [gb_s[:, 3:4]], [gb_s[:, 7:8]], a_s[:, 2:3], "hB",
                       True)

            rex = h3[0]
            for b in range(T):
                ptb = pstr.tile([P, P], BF16, tag="ptrbf", name="ptrbf")
                nc.tensor.transpose(ptb[:], rex[:, b * P:(b + 1) * P],
                                    idbf_s[:])
                rn = wk.tile([P, P], F32, tag="rn", name="rn")
                nc.vector.tensor_copy(out=rn[:], in_=ptb[:])
                nc.sync.dma_start(rex_dram[b * P:(b + 1) * P, :], rn[:])
            acc = cpool.tile([P, mc], F32, tag="acc", name="acc")
            for m in range(mc):
                pg = gp.tile([P, IN], F32, tag="pg", name="pg")
                nc.gpsimd.indirect_dma_start(
                    out=pg[:], out_offset=None, in_=rex_dram[:],
                    in_offset=IndirectOffsetOnAxis(ap=mrl_s[:, m:m + 1],
                                                   axis=0))
                tg = gp.tile([P, IN], F32, tag="tg", name="tg")
                nc.gpsimd.indirect_dma_start(
                    out=tg[:], out_offset=None, in_=x_full[:],
                    in_offset=IndirectOffsetOnAxis(ap=mrg_s[:, m:m + 1],
                                                   axis=0))
                pp = wk.tile([P, 1], F32, tag="pp", name="pp")
                tt = wk.tile([P, 1], F32, tag="tt", name="tt")
                ptv = wk.tile([P, 1], F32, tag="ptv", name="ptv")
                tmp = wk.tile([P, IN], F32, tag="tmp", name="tmp")
                nc.vector.tensor_tensor(out=tmp[:], in0=pg[:], in1=pg[:],
                                        op=OP.mult)
                nc.vector.tensor_reduce(out=pp[:], in_=tmp[:],
                                        axis=mybir.AxisListType.X, op=OP.add)
                nc.vector.tensor_tensor(out=tmp[:], in0=tg[:], in1=tg[:],
                                        op=OP.mult)
                nc.vector.tensor_reduce(out=tt[:], in_=tmp[:],
                                        axis=mybir.AxisListType.X, op=OP.add)
                nc.vector.tensor_tensor(out=tmp[:], in0=pg[:], in1=tg[:],
                                        op=OP.mult)
                nc.vector.tensor_reduce(out=ptv[:], in_=tmp[:],
                                        axis=mybir.AxisListType.X, op=OP.add)
                q = wk.tile([P, 1], F32, tag="q", name="q")
                nc.vector.tensor_tensor(out=q[:], in0=pp[:], in1=tt[:],
                                        op=OP.mult)
                nc.vector.tensor_scalar(out=q[:], in0=q[:], scalar1=1e-30,
                                        scalar2=None, op0=OP.add)
                rq = wk.tile([P, 1], F32, tag="rq", name="rq")
                nc.scalar.activation(rq[:], q[:],
                                     mybir.ActivationFunctionType.Sqrt)
                nc.vector.reciprocal(rq[:], rq[:])
                nc.vector.tensor_tensor(out=rq[:], in0=ptv[:], in1=rq[:],
                                        op=OP.mult)
                nc.vector.tensor_tensor(out=acc[:, m:m + 1], in0=rq[:],
                                        in1=mv_s[:, m:m + 1], op=OP.mult)
            accr = wk.tile([P, 1], F32, tag="accr", name="accr")
            nc.vector.tensor_reduce(out=accr[:], in_=acc[:],
                                    axis=mybir.AxisListType.X, op=OP.add)
            pl = pstr.tile([1, 1], F32, tag="ptr32", name="ptr32")
            nc.tensor.matmul(pl[:], lhsT=accr[:], rhs=ones_s[:], start=True,
                             stop=True)
            lsb = wk.tile([1, 16], F32, tag="lsb", name="lsb")
            nc.gpsimd.memset(lsb[:], 0.0)
            nc.vector.tensor_copy(out=lsb[:, 0:1], in_=pl[:])
            nc.sync.dma_start(loss_in[:], lsb[:])
            cc3 = nc.gpsimd.collective_compute(
                "AllReduce", OP.add, replica_groups=RG,
                ins=[loss_in[:].opt()], outs=[loss_out[:].opt()])
            lsum = wk.tile([1, 16], F32, tag="lsum", name="lsum")
            ld3 = nc.sync.dma_start(lsum[:], loss_out[:])
            add_dep_helper(ld3.ins, cc3.ins, True, "loss after ar")
            nc.vector.tensor_scalar(out=lsb[:, 0:1], in0=lsum[:, 0:1],
                                    scalar1=-1.0 / NMASK, scalar2=1.0,
                                    op0=OP.mult, op1=OP.add)
            nc.sync.dma_start(out_t[:], lsb[:, 0:1])
    return nc


def kernel(**inputs):
    inputs = {k: np.asarray(v) for k, v in inputs.items()}
    edge_index = inputs["edge_index"].astype(np.int64)
    mask_nodes = inputs["mask_nodes"].astype(np.int64)
    x = inputs["x"].astype(np.float32)
    (dinv_pad, srcs, dstl, kc, mlocal, mglob, mvalid, mkeep, mc) = _prep(
        edge_index, mask_nodes)

    nc = build_nc(kc, mc)

    iota = np.broadcast_to(np.arange(P, dtype=np.float32), (P, P)).astype(BF)
    ident_bf = np.eye(P, dtype=np.float32).astype(BF)
    ident_f32 = np.eye(P, dtype=np.float32)
    gb = np.zeros((P, 8), np.float32)
    gb[:, 0] = inputs["g1"][:P]
    gb[:, 1] = inputs["g1"][P:]
    gb[:, 2] = inputs["g2"]
    gb[:, 3] = inputs["gd"]
    gb[:, 4] = inputs["be1"][:P]
    gb[:, 5] = inputs["be1"][P:]
    gb[:, 6] = inputs["be2"]
    gb[:, 7] = inputs["bed"]
    a_rep = np.zeros((P, 3), np.float32)
    a_rep[:, 0] = inputs["a1"][0]
    a_rep[:, 1] = inputs["a2"][0]
    a_rep[:, 2] = inputs["ad"][0]
    w1 = inputs["W1"].astype(BF)
    w2 = inputs["W2"].astype(BF)
    wd = inputs["Wd"].astype(BF)
    token = inputs["mask_token"].astype(np.float32)

    in_maps = []
    for c in range(NC):
        rows = np.arange(c * SHARD, (c + 1) * SHARD)
        xs = np.zeros((SHARD, IN), np.float32)
        v = rows < N
        xs[v] = x[rows[v]]
        mcol = np.ascontiguousarray(
            mkeep[c].reshape(T, P).T)  # [128, T]
        in_maps.append({
            "x_shard": xs,
            "x_full": x,
            "src_rows": srcs[c],
            "dst_local": dstl[c],
            "dinv_rep": np.ascontiguousarray(np.broadcast_to(
                dinv_pad[c * SHARD:(c + 1) * SHARD][None, :], (P, SHARD))),
            "mcol": mcol,
            "token_rep": np.ascontiguousarray(
                np.broadcast_to(token[None, :], (P, IN))),
            "iota_bf": np.ascontiguousarray(iota),
            "ident_bf": ident_bf,
            "ident_f32": ident_f32,
            "ones_col": np.ones((P, 1), np.float32),
            "w1": w1,
            "w2a": np.ascontiguousarray(w2[:P]),
            "w2b": np.ascontiguousarray(w2[P:]),
            "wd": wd,
            "gb": gb,
            "a_rep": a_rep,
            "mrow_l": mlocal[c],
            "mrow_g": mglob[c],
            "mval": mvalid[c],
        })
    import os
    res = run_bass_kernel_spmd(nc, in_maps, core_ids=list(range(NC)),
                               trace=bool(os.environ.get("KTRACE")))
    kernel._last_results = res
    loss = res.results[0]["loss"][0, 0]
    return np.float32(loss).reshape(())



# revision 11
# speedup vs baseline: 8.3536x; 8.3536x over previous
"""GraphMAE-style GNN forward (3x GCNConv + BN + PReLU + SCE loss) on 8 TRN2
NeuronCores via Bass/Tile — v2.

Design (vs v1 baseline):
- Aggregate-first GCN: every layer's exchanged node table is 128-wide
  (u_k = dinv * (h_{k-1} @ W_k), with W applied before aggregation for L1's
  input side and after h for L2/L3 via the tail of the previous layer).
- Tables stored in DRAM as fp8e4m3; edge-row gathers are batched (one
  indirect DMA per GB dst tiles) and cast fp8->bf16 in the DMA (gpsimd SWDGE
  cast) — slashes both SWDGE instruction count and HBM gather bytes.
- table1 (= dinv * masked-x) is built host-side and shipped as a per-core
  input: kills AllGather #1 and the device-side masking pass entirely.
- Self-loops ride the edge list as one extra chunk per dst tile whose one-hot
  is the identity (no S build, no extra node-major copy).
- One-hot S built via tensor_scalar is_equal against an iota tile (DVE 4x
  mode), bf16.
- BN stats: per-tile free-dim sums via TTR accum + ScalarE Square accum;
  single [P, 2*nfh] f32 AllReduce per layer. Affine+PReLU as
  max(A*z+B, aA*z+aB) with the two affine maps on ScalarE, max on DVE.
- Loss: re_x written (row-permuted) to DRAM, masked rows gathered with one
  batched indirect DMA; targets x/||x|| are host-precomputed and shipped.
- DRAM row order of all tables is permuted (groups of 4 tiles interleaved)
  so agin/rex writes are 4-tile batched contiguous DMAs; gather offsets are
  host-side permuted to match.

Conv bias is skipped: training-mode BN subtracts the batch mean, so it
cancels exactly.
"""
import sys

sys.path.insert(0, "/opt/trn_rl_repo")
import numpy as np
import ml_dtypes

import concourse.bass as bass
import concourse.mybir as mybir
import concourse.tile as tile
from bass_rust import SyncInfo
from concourse.bass import IndirectOffsetOnAxis
from concourse.bass_utils import run_bass_kernel_spmd
from concourse.tile_rust import add_dep_helper
from concourse.vector_clock import ScopedClock

BF = ml_dtypes.bfloat16
F8 = ml_dtypes.float8_e4m3fn
F32 = mybir.dt.float32
BF16 = mybir.dt.bfloat16
FP8 = mybir.dt.float8e4
I32 = mybir.dt.int32
OP = mybir.AluOpType
ACT = mybir.ActivationFunctionType

N, E, IN, HID, OUT = 50000, 800000, 128, 256, 128
NC, P = 8, 128
SHARD, T = 6272, 49
NPAD = NC * SHARD
NMASK = 25000
EPS = 1e-5
RG = [list(range(NC))]
GB = 2          # dst tiles per batched gather
WGRP = 4        # tiles per batched agin/rex write (row-permutation group)

# ---------------------------------------------------------------------------
# Walrus on this toolchain rejects >1 semaphore wait per instruction
# ("Too many sync wait commands"). Post-process lowered blocks: hoist excess
# waits onto injected same-engine NoOps (program order => equivalent).
_uid = [0]


def _split_bb_waits(nc):
    for f in nc.m.functions:
        for bb in f.blocks:
            insts = list(bb.instructions)
            out = []
            changed = False
            for inst in insts:
                si = inst.sync_info
                waits = list(si.on_wait) if si is not None and si.on_wait else []
                if len(waits) > 1:
                    changed = True
                    rest = waits[:-1]
                    inst.sync_info.on_wait = waits[-1:]
                    while rest:
                        _uid[0] += 1
                        nop = mybir.InstNoOp(
                            name=f"I-waitsplit-{_uid[0]}", ins=[], outs=[])
                        nop.engine = inst.engine
                        nop.sync_info = SyncInfo(
                            on_wait=rest[:1], on_update=[])
                        rest = rest[1:]
                        out.append(nop)
                out.append(inst)
            if changed:
                bb.instructions = out


class TileContextSplitDrain(tile.TileContext):
    def _drain_and_barrier(self, tick_clock, wait_clock):
        nc = self.nc
        probe = nc.sync.nop(nofuse=True)
        wait_clock.add_sem_waits(
            probe.ins, ScopedClock({None: tick_clock.global_clock}))
        nc.sync.drain()
        nc.all_engine_barrier()
        assert self.sems is not None
        popped = nc._tile_sem_poison_stack.pop()
        assert popped is self._sem_poison
        nc.clear_and_free_semaphores(list(self.sems.allocated().values()))
        nc.all_engine_barrier()
        _split_bb_waits(nc)


# ---------------------------------------------------------------------------
def _pos_perm():
    """Row permutation within one shard: tiles grouped by WGRP, rows of a
    group interleaved so a [P, tiles_in_group*P] SBUF stage DMAs contiguously
    to DRAM rows. pos[t*P + p] = group_base + p*tig + (t - g*WGRP)."""
    pos = np.empty(SHARD, np.int64)
    for t in range(T):
        g = t // WGRP
        tig = min(WGRP, T - g * WGRP)
        base = g * WGRP * P
        for_p = base + np.arange(P) * tig + (t - g * WGRP)
        pos[t * P:(t + 1) * P] = for_p
    return pos


def _prep(x, edge_index, mask_nodes, mask_token):
    src = edge_index[0].astype(np.int64)
    dst = edge_index[1].astype(np.int64)
    deg = np.bincount(dst, minlength=N).astype(np.float64) + 1.0
    dinv = (1.0 / np.sqrt(deg)).astype(np.float32)

    mask_set = np.zeros(N, bool)
    mask_set[mask_nodes] = True
    keep = np.where(mask_set, 0.0, 1.0).astype(np.float32)

    pos = _pos_perm()
    # global table row for node n (row-permuted within its core's shard)
    nglob = np.arange(N)
    table_row = (nglob // SHARD) * SHARD + pos[nglob % SHARD]

    # table1 = dinv * x_masked, fp8, permuted rows; pad rows zero
    xh = x.copy()
    xh[mask_nodes] = mask_token
    u1 = (dinv[:, None] * xh).astype(F8)
    table1 = np.zeros((NPAD, IN), F8)
    table1[table_row] = u1

    # per-core, per-tile edge lists (dst-sharded)
    core_of = dst // SHARD
    pertile = [[None] * T for _ in range(NC)]
    cnts = np.zeros((NC, T), np.int64)
    for c in range(NC):
        sel = core_of == c
        es, ed = src[sel], dst[sel] - c * SHARD
        tl = ed // P
        order = np.argsort(tl, kind="stable")
        es, ed, tl = es[order], ed[order], tl[order]
        bounds = np.searchsorted(tl, np.arange(T + 1))
        for t in range(T):
            a, b = bounds[t], bounds[t + 1]
            pertile[c][t] = (es[a:b], (ed[a:b] % P).astype(np.float32))
            cnts[c, t] = (b - a + P - 1) // P
    kcs = cnts.max(axis=0) + 1          # +1 self chunk (last)
    cum = np.zeros(T + 1, np.int64)
    cum[1:] = np.cumsum(kcs)
    C = int(cum[-1])

    offs_l, dstl_l = [], []
    for c in range(NC):
        oc = np.zeros((C, P), np.int64)
        dc = np.full((C, P), 255.0, np.float32)
        for t in range(T):
            es, dl = pertile[c][t]
            n = len(es)
            lo = cum[t]
            oc[lo:lo + kcs[t] - 1].reshape(-1)[:n] = table_row[es]
            dc[lo:lo + kcs[t] - 1].reshape(-1)[:n] = dl
            # self chunk: the tile's own (permuted) rows, identity one-hot
            loc = t * P + np.arange(P)
            oc[cum[t + 1] - 1] = c * SHARD + pos[loc]
        offs_l.append(np.ascontiguousarray(oc.T.astype(np.int32)))
        dstl_l.append(np.ascontiguousarray(dc.T))

    # loss: masked rows per core (permuted local row ids) + normalized targets
    mloc_l, that_l = [], []
    mc = 0
    percore_masked = []
    for c in range(NC):
        lo = c * SHARD
        hi = min((c + 1) * SHARD, N)
        rows = np.arange(lo, hi)
        mm = rows[mask_set[rows]]
        percore_masked.append(mm)
        mc = max(mc, (len(mm) + P - 1) // P)
    for c in range(NC):
        mm = percore_masked[c]
        n = len(mm)
        lo = np.zeros((mc, P), np.int64)
        lo.reshape(-1)[:n] = pos[mm % SHARD]
        th = np.zeros((mc, P, IN), np.float32)
        tv = x[mm]
        nr = np.linalg.norm(tv, axis=1)
        tv = tv / np.maximum(nr, 1e-12)[:, None]
        th.reshape(-1, IN)[:n] = tv
        mloc_l.append(np.ascontiguousarray(lo.T.astype(np.int32)))
        # [P, mc, IN] -> [P, mc*IN]
        that_l.append(np.ascontiguousarray(
            th.transpose(1, 0, 2).reshape(P, mc * IN).astype(BF)))

    dinv_pad = np.zeros(NPAD, np.float32)
    dinv_pad[:N] = dinv
    keep_pad = np.zeros(NPAD, np.float32)
    keep_pad[:N] = keep
    return (dinv_pad, keep_pad, table1, offs_l, dstl_l, kcs, cum, C,
            mloc_l, that_l, mc)


# ---------------------------------------------------------------------------
def build_nc(kcs, cum, C, mc, alphas):
    a1, a2, ad = alphas
    nc = bass.Bass(num_devices=NC, target_bir_lowering=False)
    D = {}

    def inp(name, shape, dt):
        D[name] = nc.dram_tensor(name, shape, dt, kind="ExternalInput")
        return D[name]

    inp("table1", [NPAD, IN], FP8)
    inp("offs", [P, C], I32)
    inp("dstl", [P, C], F32)
    inp("dinv_rep", [P, SHARD], BF16)
    inp("dinvm_rep", [P, SHARD], BF16)
    inp("iota_bf", [P, P], BF16)
    inp("ident_bf", [P, P], BF16)
    inp("ones_col", [P, 1], F32)
    inp("w1", [IN, HID], BF16)
    inp("w2a", [P, OUT], BF16)
    inp("w2b", [P, OUT], BF16)
    inp("wd", [OUT, IN], BF16)
    inp("gb", [P, 8], F32)
    inp("mloc", [P, mc], I32)
    inp("that", [P, mc * IN], BF16)
    out_t = nc.dram_tensor("loss", [1, 1], F32, kind="ExternalOutput")

    GMAXW = max(int(kcs[t] + (kcs[t + 1] if t + 1 < T else 0))
                for t in range(0, T, GB))

    with TileContextSplitDrain(nc) as tc:
        with (
            tc.tile_pool(name="const", bufs=1) as cpool,
            tc.tile_pool(name="hbuf", bufs=1) as hpool,
            tc.tile_pool(name="zbuf", bufs=1) as zpool,
            tc.tile_pool(name="work", bufs=2) as wk,
            tc.tile_pool(name="spool", bufs=3) as spool,
            tc.tile_pool(name="gpool", bufs=3) as gp,
            tc.tile_pool(name="stage", bufs=2) as stg,
            tc.tile_pool(name="psagg", bufs=2, space="PSUM") as psagg,
            tc.tile_pool(name="psmm", bufs=2, space="PSUM") as psmm,
            tc.tile_pool(name="pstr", bufs=2, space="PSUM") as pstr,
            tc.tile_pool(name="psl", bufs=1, space="PSUM") as psl,
            tc.tile_pool(name="dram", bufs=1, space="DRAM") as dpool,
        ):
            def load(tname):
                h = D[tname]
                t_ = cpool.tile(list(h.shape), h.dtype, tag=tname, name=tname)
                nc.sync.dma_start(t_[:], h[:])
                return t_

            offs_s = load("offs")
            dstl_s = load("dstl")
            dinv_s = load("dinv_rep")
            dinvm_s = load("dinvm_rep")
            iota_s = load("iota_bf")
            idbf_s = load("ident_bf")
            ones_s = load("ones_col")
            w1_s = load("w1")
            w2a_s = load("w2a")
            w2b_s = load("w2b")
            wd_s = load("wd")
            gb_s = load("gb")
            mloc_s = load("mloc")
            that_s = load("that")

            table2 = dpool.tile([NPAD, OUT], FP8, addr_space="Shared",
                                name="table2")
            table3 = dpool.tile([NPAD, IN], FP8, addr_space="Shared",
                                name="table3")
            agin2 = dpool.tile([SHARD, OUT], FP8, name="agin2")
            agin3 = dpool.tile([SHARD, IN], FP8, name="agin3")
            rex_dram = dpool.tile([SHARD, IN], BF16, name="rex_dram")
            loss_in = dpool.tile([1, 16], F32, name="loss_in")
            loss_out = dpool.tile([1, 16], F32, addr_space="Shared",
                                  name="loss_out")

            def gather_groups(table_h, lname):
                """Issue batched gathers; return list of (t0, nt, colbase)
                plus the G tiles in group order."""
                gts = []
                for t0 in range(0, T, GB):
                    nt = min(GB, T - t0)
                    w = int(cum[t0 + nt] - cum[t0])
                    G = gp.tile([P, GMAXW * P], FP8, tag="G", name=f"G_{lname}")
                    nc.gpsimd.indirect_dma_start(
                        out=G[:, :w * P], out_offset=None, in_=table_h[:],
                        in_offset=IndirectOffsetOnAxis(
                            ap=offs_s[:, int(cum[t0]):int(cum[t0]) + w],
                            axis=0))
                    gts.append((t0, nt, G))
                return gts

            def agg_chunks(G, t, colbase, pa):
                """Accumulate aggregation matmuls for dst tile t into pa."""
                kk = int(kcs[t])
                for k in range(kk):
                    col = int(cum[t]) + k
                    gsl = G[:, (colbase + k) * P:(colbase + k + 1) * P]
                    if k == kk - 1:
                        rhs = idbf_s[:]
                    else:
                        S = spool.tile([P, P], BF16, tag="S", name="S")
                        nc.vector.tensor_scalar(
                            out=S[:], in0=iota_s[:],
                            scalar1=dstl_s[:, col:col + 1], scalar2=None,
                            op0=OP.is_equal)
                        rhs = S[:]
                    nc.tensor.matmul(pa[:], lhsT=gsl, rhs=rhs,
                                     start=(k == 0), stop=(k == kk - 1))

            def stats_ar(s_cols, q_cols, nfh, lname):
                """Reduce per-tile stat columns, AllReduce, return A/B/A2/B2
                column tiles per fh."""
                st = wk.tile([P, 2 * nfh], F32, tag="st", name=f"st_{lname}")
                for fh in range(nfh):
                    nc.vector.tensor_reduce(
                        out=st[:, fh:fh + 1], in_=s_cols[fh][:],
                        axis=mybir.AxisListType.X, op=OP.add)
                    nc.vector.tensor_reduce(
                        out=st[:, nfh + fh:nfh + fh + 1], in_=q_cols[fh][:],
                        axis=mybir.AxisListType.X, op=OP.add)
                sin = dpool.tile([P, 2 * nfh], F32, name=f"stats_in_{lname}")
                sout = dpool.tile([P, 2 * nfh], F32, addr_space="Shared",
                                  name=f"stats_out_{lname}")
                nc.sync.dma_start(sin[:], st[:])
                cc = nc.gpsimd.collective_compute(
                    "AllReduce", OP.add, replica_groups=RG,
                    ins=[sin[:].opt()], outs=[sout[:].opt()])
                st2 = wk.tile([P, 2 * nfh], F32, tag="st2", name=f"st2_{lname}")
                ld = nc.sync.dma_start(st2[:], sout[:])
                add_dep_helper(ld.ins, cc.ins, True, "stats after ar")
                return st2

            def make_ab(st2, nfh, g_cols, be_cols, alpha, lname):
                AB = []
                for fh in range(nfh):
                    mu = wk.tile([P, 1], F32, tag=f"mu{fh}", name=f"mu_{lname}")
                    nc.vector.tensor_scalar(out=mu[:], in0=st2[:, fh:fh + 1],
                                            scalar1=1.0 / N, scalar2=None,
                                            op0=OP.mult)
                    ms = wk.tile([P, 1], F32, tag=f"ms{fh}", name=f"ms_{lname}")
                    nc.vector.tensor_scalar(
                        out=ms[:], in0=st2[:, nfh + fh:nfh + fh + 1],
                        scalar1=1.0 / N, scalar2=None, op0=OP.mult)
                    var = wk.tile([P, 1], F32, tag=f"var{fh}", name=f"var_{lname}")
                    nc.vector.tensor_tensor(out=var[:], in0=mu[:], in1=mu[:],
                                            op=OP.mult)
                    nc.vector.tensor_tensor(out=var[:], in0=ms[:], in1=var[:],
                                            op=OP.subtract)
                    nc.vector.tensor_scalar(out=var[:], in0=var[:], scalar1=EPS,
                                            scalar2=None, op0=OP.add)
                    rs = wk.tile([P, 1], F32, tag=f"rs{fh}", name=f"rs_{lname}")
                    nc.scalar.activation(rs[:], var[:], ACT.Sqrt)
                    nc.vector.reciprocal(rs[:], rs[:])
                    A = wk.tile([P, 1], F32, tag=f"A{fh}", name=f"A_{lname}")
                    nc.vector.tensor_tensor(out=A[:], in0=g_cols[fh], in1=rs[:],
                                            op=OP.mult)
                    Bv = wk.tile([P, 1], F32, tag=f"B{fh}", name=f"B_{lname}")
                    nc.vector.tensor_tensor(out=Bv[:], in0=mu[:], in1=A[:],
                                            op=OP.mult)
                    nc.vector.tensor_tensor(out=Bv[:], in0=be_cols[fh],
                                            in1=Bv[:], op=OP.subtract)
                    A2 = wk.tile([P, 1], F32, tag=f"A2{fh}", name=f"A2_{lname}")
                    nc.vector.tensor_scalar(out=A2[:], in0=A[:], scalar1=alpha,
                                            scalar2=None, op0=OP.mult)
                    B2 = wk.tile([P, 1], F32, tag=f"B2{fh}", name=f"B2_{lname}")
                    nc.vector.tensor_scalar(out=B2[:], in0=Bv[:], scalar1=alpha,
                                            scalar2=None, op0=OP.mult)
                    AB.append((A, Bv, A2, B2))
                return AB

            def affine_prelu(z_tiles, AB, h_tiles, t):
                sl = slice(t * P, (t + 1) * P)
                for fh in range(len(z_tiles)):
                    A, Bv, A2, B2 = AB[fh]
                    t1 = wk.tile([P, P], BF16, tag="t1", name="t1")
                    nc.scalar.activation(t1[:], z_tiles[fh][:, sl],
                                         ACT.Identity,
                                         bias=Bv[:, 0:1], scale=A[:, 0:1])
                    t2 = wk.tile([P, P], BF16, tag="t2", name="t2")
                    nc.scalar.activation(t2[:], z_tiles[fh][:, sl],
                                         ACT.Identity,
                                         bias=B2[:, 0:1], scale=A2[:, 0:1])
                    nc.vector.tensor_tensor(out=h_tiles[fh][:, sl], in0=t1[:],
                                            in1=t2[:], op=OP.max)

            def pack_into(stage, src_bf_tile, j):
                """Transpose f-major tile -> node-major block j of the
                group staging tile (row permutation makes the group DMA
                contiguous in DRAM)."""
                pt = pstr.tile([P, P], BF16, tag="ptr", name="ptr")
                nc.tensor.transpose(pt[:], src_bf_tile, idbf_s[:])
                nc.scalar.activation(
                    stage[:, j * P:(j + 1) * P], pt[:], ACT.Copy)

            # ---------------- layer 1 (nfh=2) ----------------
            z1 = [zpool.tile([P, SHARD], BF16, tag=f"z_{fh}", name=f"z1_{fh}")
                  for fh in range(2)]
            s_cols = [wk.tile([P, T], F32, tag=f"sc{fh}", name=f"sc1_{fh}")
                      for fh in range(2)]
            q_cols = [wk.tile([P, T], F32, tag=f"qc{fh}", name=f"qc1_{fh}")
                      for fh in range(2)]
            for (t0, nt, G) in gather_groups(D["table1"], "L1"):
                for ti in range(nt):
                    t = t0 + ti
                    colbase = int(cum[t] - cum[t0])
                    sl = slice(t * P, (t + 1) * P)
                    pa = psagg.tile([P, P], F32, tag="pa", name="pa1")
                    agg_chunks(G, t, colbase, pa)
                    y = wk.tile([P, P], BF16, tag="y", name="y1")
                    nc.vector.tensor_tensor(out=y[:], in0=pa[:],
                                            in1=dinv_s[:, sl], op=OP.mult)
                    for fh in range(2):
                        zp = psmm.tile([P, P], F32, tag="mm", name="zp1")
                        nc.tensor.matmul(
                            zp[:], lhsT=w1_s[:, fh * P:(fh + 1) * P],
                            rhs=y[:], start=True, stop=True)
                        nc.scalar.activation(
                            z1[fh][:, sl], zp[:], ACT.Copy,
                            accum_out=s_cols[fh][:, t:t + 1])
                        scr = wk.tile([P, P], BF16, tag="scr", name="scr1")
                        nc.scalar.activation(
                            scr[:], zp[:], ACT.Square,
                            accum_out=q_cols[fh][:, t:t + 1])
            st2 = stats_ar(s_cols, q_cols, 2, "L1")
            AB1 = make_ab(st2, 2, [gb_s[:, 0:1], gb_s[:, 1:2]],
                          [gb_s[:, 4:5], gb_s[:, 5:6]], a1, "L1")
            # tail: h1 = prelu(affine(z1)); u2 = dinv*(h1@W2) -> agin2
            h1 = [hpool.tile([P, SHARD], BF16, tag=f"hA_{fh}",
                             name=f"h1_{fh}") for fh in range(2)]
            for g in range(0, T, WGRP):
                tig = min(WGRP, T - g)
                stage = stg.tile([P, WGRP * P], FP8, tag="stg8", name="stg2")
                for j in range(tig):
                    t = g + j
                    sl = slice(t * P, (t + 1) * P)
                    affine_prelu(z1, AB1, h1, t)
                    up = psmm.tile([P, P], F32, tag="mm", name="up2")
                    nc.tensor.matmul(up[:], lhsT=w2a_s[:], rhs=h1[0][:, sl],
                                     start=True, stop=False)
                    nc.tensor.matmul(up[:], lhsT=w2b_s[:], rhs=h1[1][:, sl],
                                     start=False, stop=True)
                    ub = wk.tile([P, P], BF16, tag="ub", name="ub2")
                    nc.vector.tensor_tensor(out=ub[:], in0=up[:],
                                            in1=dinv_s[:, sl], op=OP.mult)
                    pack_into(stage, ub[:], j)
                nc.sync.dma_start(
                    agin2[g * P:(g + tig) * P, :], stage[:, :tig * P])
            cc2 = nc.gpsimd.collective_compute(
                "AllGather", OP.bypass, replica_groups=RG,
                ins=[agin2[:].opt()], outs=[table2[:].opt()])
            fence2 = nc.gpsimd.memset(
                wk.tile([1, 1], F32, tag="fence", name="fence2")[:], 0.0)
            add_dep_helper(fence2.ins, cc2.ins, True, "fence cc2")

            # ---------------- layer 2 (nfh=1) ----------------
            z2 = [zpool.tile([P, SHARD], BF16, tag="z_0", name="z2")]
            s_cols = [wk.tile([P, T], F32, tag="sc0", name="sc2")]
            q_cols = [wk.tile([P, T], F32, tag="qc0", name="qc2")]
            for (t0, nt, G) in gather_groups(table2, "L2"):
                for ti in range(nt):
                    t = t0 + ti
                    colbase = int(cum[t] - cum[t0])
                    sl = slice(t * P, (t + 1) * P)
                    pa = psagg.tile([P, P], F32, tag="pa", name="pa2")
                    agg_chunks(G, t, colbase, pa)
                    scr = wk.tile([P, P], BF16, tag="scr", name="scr2")
                    nc.vector.tensor_tensor(
                        out=z2[0][:, sl], in0=pa[:], in1=dinv_s[:, sl],
                        op=OP.mult)
                    nc.vector.tensor_reduce(
                        out=s_cols[0][:, t:t + 1], in_=z2[0][:, sl],
                        axis=mybir.AxisListType.X, op=OP.add)
                    nc.scalar.activation(
                        scr[:], z2[0][:, sl], ACT.Square,
                        accum_out=q_cols[0][:, t:t + 1])
            st2b = stats_ar(s_cols, q_cols, 1, "L2")
            AB2 = make_ab(st2b, 1, [gb_s[:, 2:3]], [gb_s[:, 6:7]], a2, "L2")
            h2 = [hpool.tile([P, SHARD], BF16, tag="hB_0", name="h2")]
            for g in range(0, T, WGRP):
                tig = min(WGRP, T - g)
                stage = stg.tile([P, WGRP * P], FP8, tag="stg8", name="stg3")
                for j in range(tig):
                    t = g + j
                    sl = slice(t * P, (t + 1) * P)
                    affine_prelu(z2, AB2, h2, t)
                    up = psmm.tile([P, P], F32, tag="mm", name="up3")
                    nc.tensor.matmul(up[:], lhsT=wd_s[:], rhs=h2[0][:, sl],
                                     start=True, stop=True)
                    ub = wk.tile([P, P], BF16, tag="ub", name="ub3")
                    nc.vector.tensor_tensor(out=ub[:], in0=up[:],
                                            in1=dinvm_s[:, sl], op=OP.mult)
                    pack_into(stage, ub[:], j)
                nc.sync.dma_start(
                    agin3[g * P:(g + tig) * P, :], stage[:, :tig * P])
            cc3 = nc.gpsimd.collective_compute(
                "AllGather", OP.bypass, replica_groups=RG,
                ins=[agin3[:].opt()], outs=[table3[:].opt()])
            fence3 = nc.gpsimd.memset(
                wk.tile([1, 1], F32, tag="fence", name="fence3")[:], 0.0)
            add_dep_helper(fence3.ins, cc3.ins, True, "fence cc3")

            # ---------------- layer 3 (nfh=1) ----------------
            z3 = [zpool.tile([P, SHARD], BF16, tag="z_1", name="z3")]
            s_cols = [wk.tile([P, T], F32, tag="sc0", name="sc3")]
            q_cols = [wk.tile([P, T], F32, tag="qc0", name="qc3")]
            for (t0, nt, G) in gather_groups(table3, "L3"):
                for ti in range(nt):
                    t = t0 + ti
                    colbase = int(cum[t] - cum[t0])
                    sl = slice(t * P, (t + 1) * P)
                    pa = psagg.tile([P, P], F32, tag="pa", name="pa3")
                    agg_chunks(G, t, colbase, pa)
                    scr = wk.tile([P, P], BF16, tag="scr", name="scr3")
                    nc.vector.tensor_tensor(
                        out=z3[0][:, sl], in0=pa[:], in1=dinv_s[:, sl],
                        op=OP.mult)
                    nc.vector.tensor_reduce(
                        out=s_cols[0][:, t:t + 1], in_=z3[0][:, sl],
                        axis=mybir.AxisListType.X, op=OP.add)
                    nc.scalar.activation(
                        scr[:], z3[0][:, sl], ACT.Square,
                        accum_out=q_cols[0][:, t:t + 1])
            st2c = stats_ar(s_cols, q_cols, 1, "L3")
            AB3 = make_ab(st2c, 1, [gb_s[:, 3:4]], [gb_s[:, 7:8]], ad, "L3")
            h3 = [hpool.tile([P, SHARD], BF16, tag="hA_0", name="h3")]
            for g in range(0, T, WGRP):
                tig = min(WGRP, T - g)
                stage = stg.tile([P, WGRP * P], BF16, tag="stgr", name="stgr")
                for j in range(tig):
                    t = g + j
                    sl = slice(t * P, (t + 1) * P)
                    affine_prelu(z3, AB3, h3, t)
                    pack_into(stage, h3[0][:, sl], j)
                nc.sync.dma_start(
                    rex_dram[g * P:(g + tig) * P, :], stage[:, :tig * P])

            # ---------------- loss ----------------
            pg = gp.tile([P, mc * IN], BF16, tag="pg", name="pg")
            nc.gpsimd.indirect_dma_start(
                out=pg[:], out_offset=None, in_=rex_dram[:],
                in_offset=IndirectOffsetOnAxis(ap=mloc_s[:], axis=0))
            num_c = wk.tile([P, mc], F32, tag="numc", name="numc")
            pp_c = wk.tile([P, mc], F32, tag="ppc", name="ppc")
            for m in range(mc):
                msl = slice(m * IN, (m + 1) * IN)
                wscr = wk.tile([P, IN], BF16, tag="wscr", name="wscr")
                nc.vector.tensor_tensor(
                    out=wscr[:], in0=pg[:, msl], in1=that_s[:, msl],
                    op=OP.mult)
                nc.vector.tensor_reduce(
                    out=num_c[:, m:m + 1], in_=wscr[:],
                    axis=mybir.AxisListType.X, op=OP.add)
                wscr2 = wk.tile([P, IN], BF16, tag="wscr2", name="wscr2")
                nc.vector.tensor_tensor(
                    out=wscr2[:], in0=pg[:, msl], in1=pg[:, msl],
                    op=OP.mult)
                nc.vector.tensor_reduce(
                    out=pp_c[:, m:m + 1], in_=wscr2[:],
                    axis=mybir.AxisListType.X, op=OP.add)
            nc.vector.tensor_scalar(out=pp_c[:], in0=pp_c[:], scalar1=1e-12,
                                    scalar2=None, op0=OP.add)
            rr = wk.tile([P, mc], F32, tag="rr", name="rr")
            nc.scalar.activation(rr[:], pp_c[:], ACT.Sqrt)
            nc.vector.reciprocal(rr[:], rr[:])
            nc.vector.tensor_tensor(out=rr[:], in0=num_c[:], in1=rr[:],
                                    op=OP.mult)
            accr = wk.tile([P, 1], F32, tag="accr", name="accr")
            nc.vector.tensor_reduce(out=accr[:], in_=rr[:],
                                    axis=mybir.AxisListType.X, op=OP.add)
            pl = psl.tile([1, 1], F32, tag="pl", name="pl")
            nc.tensor.matmul(pl[:], lhsT=accr[:], rhs=ones_s[:], start=True,
                             stop=True)
            lsb = wk.tile([1, 16], F32, tag="lsb", name="lsb")
            nc.gpsimd.memset(lsb[:], 0.0)
            nc.vector.tensor_copy(out=lsb[:, 0:1], in_=pl[:])
            nc.sync.dma_start(loss_in[:], lsb[:])
            cc4 = nc.gpsimd.collective_compute(
                "AllReduce", OP.add, replica_groups=RG,
                ins=[loss_in[:].opt()], outs=[loss_out[:].opt()])
            lsum = wk.tile([1, 16], F32, tag="lsum", name="lsum")
            ld4 = nc.sync.dma_start(lsum[:], loss_out[:])
            add_dep_helper(ld4.ins, cc4.ins, True, "loss after ar")
            nc.vector.tensor_scalar(out=lsb[:, 0:1], in0=lsum[:, 0:1],
                                    scalar1=-1.0 / NMASK, scalar2=1.0,
                                    op0=OP.mult, op1=OP.add)
            nc.sync.dma_start(out_t[:], lsb[:, 0:1])
    return nc


# ---------------------------------------------------------------------------
def kernel(**inputs):
    inputs = {k: np.asarray(v) for k, v in inputs.items()}
    x = inputs["x"].astype(np.float32)
    (dinv_pad, keep_pad, table1, offs_l, dstl_l, kcs, cum, C,
     mloc_l, that_l, mc) = _prep(
        x, inputs["edge_index"].astype(np.int64),
        inputs["mask_nodes"].astype(np.int64),
        inputs["mask_token"].astype(np.float32))

    alphas = (float(inputs["a1"][0]), float(inputs["a2"][0]),
              float(inputs["ad"][0]))
    nc = build_nc(kcs, cum, C, mc, alphas)

    iota = np.broadcast_to(np.arange(P, dtype=np.float32), (P, P)).astype(BF)
    ident_bf = np.eye(P, dtype=np.float32).astype(BF)
    gbv = np.zeros((P, 8), np.float32)
    gbv[:, 0] = inputs["g1"][:P]
    gbv[:, 1] = inputs["g1"][P:]
    gbv[:, 2] = inputs["g2"]
    gbv[:, 3] = inputs["gd"]
    gbv[:, 4] = inputs["be1"][:P]
    gbv[:, 5] = inputs["be1"][P:]
    gbv[:, 6] = inputs["be2"]
    gbv[:, 7] = inputs["bed"]
    w1 = inputs["W1"].astype(BF)
    w2 = inputs["W2"].astype(BF)
    wd = inputs["Wd"].astype(BF)

    in_maps = []
    for c in range(NC):
        dsl = slice(c * SHARD, (c + 1) * SHARD)
        in_maps.append({
            "table1": table1,
            "offs": offs_l[c],
            "dstl": dstl_l[c],
            "dinv_rep": np.ascontiguousarray(np.broadcast_to(
                dinv_pad[dsl][None, :], (P, SHARD))).astype(BF),
            "dinvm_rep": np.ascontiguousarray(np.broadcast_to(
                (dinv_pad[dsl] * keep_pad[dsl])[None, :],
                (P, SHARD))).astype(BF),
            "iota_bf": np.ascontiguousarray(iota),
            "ident_bf": ident_bf,
            "ones_col": np.ones((P, 1), np.float32),
            "w1": w1,
            "w2a": np.ascontiguousarray(w2[:P]),
            "w2b": np.ascontiguousarray(w2[P:]),
            "wd": wd,
            "gb": gbv,
            "mloc": mloc_l[c],
            "that": that_l[c],
        })
    import os
    res = run_bass_kernel_spmd(nc, in_maps, core_ids=list(range(NC)),
                               trace=bool(os.environ.get("KTRACE")))
    kernel._last_results = res
    loss = res.results[0]["loss"][0, 0]
    return np.float32(loss).reshape(())
